# revision 1
# baseline (speedup 1.0000x reference)
"""Trainium2 Bass kernel for a device-aware top-1 MoE layer.

Strategy (expert parallelism over 8 NeuronCores):
  - Host: compute gate logits + top-1 routing (this is the "dispatch"
    step of the sharding), gather each expert's tokens, pad to a common
    capacity C, and transpose to feature-major [D, C] so the device
    matmuls need no on-chip transposes.
  - Device (SPMD, one NEFF on 8 cores): core i holds experts (2i, 2i+1)
    in bf16. For each expert:  hT = relu(w1.T-chunks @ xT + b1),
    yT = w2.T-chunks @ hT + b2, with fp32 PSUM accumulation.
    Activations stay [feature, token] so biases are per-partition.
  - Host: scatter each expert's [D, count] output back to token rows.

Perf notes:
  - Weights are bf16 (fp32 matmul is quarter-rate on the PE and doubles
    HBM traffic; fp32 PSUM accumulation keeps rel err ~3e-3).
  - The kernel is HBM-bound (~18 MB/core at ~360 GB/s/core). All weight
    DMA rides the sync HWDGE queue in exact consumption order; stage 2
    iterates h-outer so the PE consumes w2 tiles as they arrive instead
    of waiting for the whole expert.
  - Bias+relu / bias+copy epilogues alternate between ScalarE and
    VectorE so neither engine becomes the drain bottleneck.
"""

import numpy as np
import ml_dtypes

D = 1024
H = 2048
E = 16
NCORES = 8
P = 128
DB = D // P   # 8 d-chunks
HB = H // P   # 16 h-chunks

_program_cache = {}


def _build_program(C):
    """Trace the per-core Bass/Tile program for token capacity C (<=512)."""
    import concourse.tile as tile
    from concourse import bacc, mybir

    assert C <= 512
    f32 = mybir.dt.float32
    bf16 = mybir.dt.bfloat16
    AF = mybir.ActivationFunctionType
    ALU = mybir.AluOpType

    nc = bacc.Bacc(
        "TRN2", target_bir_lowering=False, debug=False, num_devices=NCORES
    )
    xT = nc.dram_tensor("xT", [D, 2 * C], bf16, kind="ExternalInput").ap()
    w1s = nc.dram_tensor("w1s", [2, D, H], bf16, kind="ExternalInput").ap()
    w2s = nc.dram_tensor("w2s", [2, H, D], bf16, kind="ExternalInput").ap()
    b1s = nc.dram_tensor("b1s", [2, P, HB], f32, kind="ExternalInput").ap()
    b2s = nc.dram_tensor("b2s", [2, P, DB], f32, kind="ExternalInput").ap()
    yT = nc.dram_tensor("yT", [2, D, C], bf16, kind="ExternalOutput").ap()

    with tile.TileContext(nc) as tc:
        with (
            tc.tile_pool(name="xp", bufs=2) as xp,
            tc.tile_pool(name="w1p", bufs=8) as w1p,
            tc.tile_pool(name="w2p", bufs=8) as w2p,
            tc.tile_pool(name="hp", bufs=32) as hp,
            tc.tile_pool(name="bp", bufs=4) as bp,
            tc.tile_pool(name="yp", bufs=8) as yp,
            tc.tile_pool(name="ps", bufs=8, space="PSUM") as ps,
        ):
            xts = [None, None]
            hts = [[None] * HB for _ in range(2)]
            yts = []
            b1ts = [None, None]
            b2ts = [None, None]

            # Input DMAs in consumption order on the sync HWDGE queue,
            # batched into ~1-2MB transfers (fewer triggers, deeper
            # in-flight pipelining). Tiny bias tiles go via gpsimd.
            xT3 = xT.rearrange("(o p) c -> p o c", p=P)        # [128, 8, 2C]
            w13 = [
                w1s[e].rearrange("(o p) h -> p o h", p=P) for e in range(2)
            ]                                                   # [128, 8, H]
            w23 = [
                w2s[e].rearrange("(o p) f -> p o f", p=P) for e in range(2)
            ]                                                   # [128, 16, D]

            for e in range(2):
                b1t = bp.tile([P, HB], f32, tag="b1")
                nc.gpsimd.dma_start(b1t[:], b1s[e])
                b1ts[e] = b1t
                b2t = bp.tile([P, DB], f32, tag="b2")
                nc.gpsimd.dma_start(b2t[:], b2s[e])
                b2ts[e] = b2t

            def epilogue(i, out_t, acc_t, bias_col, relu):
                """Bias (+relu) from PSUM to SBUF, alternating engines."""
                if i % 2 == 0:
                    nc.scalar.activation(
                        out_t[:], acc_t[:],
                        AF.Relu if relu else AF.Identity,
                        bias=bias_col,
                    )
                elif relu:
                    nc.vector.tensor_scalar(
                        out_t[:], acc_t[:], bias_col, 0.0, ALU.add, ALU.max
                    )
                else:
                    nc.vector.tensor_scalar_add(out_t[:], acc_t[:], bias_col)

            W1G = 2   # d-chunks per w1 DMA (1MB)
            W2G = 4   # h-chunks per w2 DMA (1MB)
            for e in range(2):
                # xT + w1 for this expert (queue position: after the
                # previous expert's w2, matching PE consumption order).
                xt = xp.tile([P, DB, C], bf16, tag="xT")
                nc.sync.dma_start(xt[:], xT3[:, :, e * C:(e + 1) * C])
                xts[e] = xt
                w1ts = []
                for g in range(DB // W1G):
                    w1t = w1p.tile([P, W1G, H], bf16, tag="w1")
                    nc.sync.dma_start(
                        w1t[:], w13[e][:, g * W1G:(g + 1) * W1G, :]
                    )
                    w1ts.append(w1t)

                # ---- stage 1: hT = relu(w1.T @ xT + b1) ----
                if e == 0:
                    # Expert 0's w1 load gates the PE start: two half-H
                    # passes, d-outer within each, so the PE starts on the
                    # first w1 d-chunks as soon as their DMAs land (the
                    # second pass reuses the then-resident w1 tiles).
                    for half in range(2):
                        accs1 = [
                            ps.tile([P, C], f32, tag="acc",
                                    name=f"acc1_{e}_{half}_{i}")
                            for i in range(HB // 2)
                        ]
                        for d in range(DB):
                            for hh in range(HB // 2):
                                h = half * (HB // 2) + hh
                                nc.tensor.matmul(
                                    accs1[hh][:],
                                    lhsT=w1ts[d // W1G][:, d % W1G, h * P:(h + 1) * P],
                                    rhs=xts[e][:, d, :],
                                    start=(d == 0),
                                    stop=(d == DB - 1),
                                )
                        for hh in range(HB // 2):
                            h = half * (HB // 2) + hh
                            ht = hp.tile([P, C], bf16, tag="hT")
                            epilogue(
                                h, ht, accs1[hh], b1ts[e][:, h:h + 1],
                                relu=True,
                            )
                            hts[e][h] = ht
                else:
                    # Expert 1's w1 is resident by the time the PE gets
                    # here: h-outer retires each psum right away, so the
                    # epilogues pipeline with the next chain's matmuls.
                    for h in range(HB):
                        acc = ps.tile([P, C], f32, tag="acc")
                        for d in range(DB):
                            nc.tensor.matmul(
                                acc[:],
                                lhsT=w1ts[d // W1G][:, d % W1G, h * P:(h + 1) * P],
                                rhs=xts[e][:, d, :],
                                start=(d == 0),
                                stop=(d == DB - 1),
                            )
                        ht = hp.tile([P, C], bf16, tag="hT")
                        epilogue(h, ht, acc, b1ts[e][:, h:h + 1], relu=True)
                        hts[e][h] = ht

                # ---- stage 2: yT = w2.T @ hT + b2 (h-outer so the PE
                # consumes each w2 tile as soon as its DMA lands) ----
                accs = [
                    ps.tile([P, C], f32, tag="acc", name=f"acc2_{e}_{d}")
                    for d in range(DB)
                ]
                for g in range(HB // W2G):
                    w2t = w2p.tile([P, W2G, D], bf16, tag="w2")
                    nc.sync.dma_start(
                        w2t[:], w23[e][:, g * W2G:(g + 1) * W2G, :]
                    )
                    for hh in range(W2G):
                        h = g * W2G + hh
                        for d in range(DB):
                            nc.tensor.matmul(
                                accs[d][:],
                                lhsT=w2t[:, hh, d * P:(d + 1) * P],
                                rhs=hts[e][h][:],
                                start=(h == 0),
                                stop=(h == HB - 1),
                            )
                yt = yp.tile([P, DB, C], bf16, tag="yt")
                for d in range(DB):
                    epilogue(
                        d, yt[:, d, :], accs[d], b2ts[e][:, d:d + 1],
                        relu=False,
                    )
                yts.append((e, yt))

            # Output writes LAST on the sync queue: the 8 HWDGE queue
            # semaphores are shared across engines, so a compute-gated
            # write queued before a weight load would head-of-line block
            # the load's trigger pacing. One batched DMA per expert.
            yT3 = yT.rearrange("e (o p) c -> e p o c", p=P)
            for e, yt in yts:
                (nc.gpsimd if e == 0 else nc.sync).dma_start(yT3[e], yt[:])

    nc.compile()
    return nc


def kernel(x, gate_w, gate_b, w1, b1, w2, b2, _trace=False):
    from concourse.bass_utils import run_bass_kernel_spmd

    x = np.asarray(x, dtype=np.float32)
    B, S, d_in = x.shape
    T = B * S
    xf = x.reshape(T, d_in)

    # --- routing (host side: this is the dispatch/sharding step) ---
    logits = xf @ np.asarray(gate_w, dtype=np.float32) + np.asarray(
        gate_b, dtype=np.float32
    )
    top1 = np.argmax(logits, axis=-1)
    idxs = [np.nonzero(top1 == e)[0] for e in range(E)]
    C = max(32, max(len(i) for i in idxs))
    C = (C + 3) // 4 * 4
    C = min(C, 512)
    assert all(len(i) <= C for i in idxs), "expert capacity overflow"

    if C not in _program_cache:
        _program_cache[C] = _build_program(C)
    nc = _program_cache[C]

    bf16 = ml_dtypes.bfloat16
    w1 = np.asarray(w1)
    w2 = np.asarray(w2)
    b1 = np.asarray(b1, dtype=np.float32)
    b2 = np.asarray(b2, dtype=np.float32)

    in_maps = []
    for core in range(NCORES):
        xT = np.zeros((D, 2 * C), dtype=bf16)
        w1s = np.empty((2, D, H), dtype=bf16)
        w2s = np.empty((2, H, D), dtype=bf16)
        b1s = np.empty((2, P, HB), dtype=np.float32)
        b2s = np.empty((2, P, DB), dtype=np.float32)
        for s in range(2):
            e = 2 * core + s
            idx = idxs[e]
            if len(idx):
                xT[:, s * C:s * C + len(idx)] = xf[idx].T.astype(bf16)
            w1s[s] = w1[e].astype(bf16)
            w2s[s] = w2[e].astype(bf16)
            b1s[s] = b1[e].reshape(HB, P).T
            b2s[s] = b2[e].reshape(DB, P).T
        in_maps.append(
            {"xT": xT, "w1s": w1s, "w2s": w2s, "b1s": b1s, "b2s": b2s}
        )

    res = run_bass_kernel_spmd(
        nc, in_maps, core_ids=list(range(NCORES)), trace=_trace
    )

    out = np.zeros((T, D), dtype=np.float32)
    for core in range(NCORES):
        yT_out = res.results[core]["yT"]
        for s in range(2):
            e = 2 * core + s
            idx = idxs[e]
            if len(idx):
                out[idx] = yT_out[s][:, :len(idx)].T.astype(np.float32)
    if _trace:
        kernel.last_result = res
    return out.reshape(B, S, D)



# revision 7
# speedup vs baseline: 1.4372x; 1.4372x over previous
"""Trainium2 Bass kernel for a device-aware top-1 MoE layer.

Strategy (expert parallelism over 8 NeuronCores):
  - Host: gate + top-1 routing, gather each expert's tokens. Experts are
    paired big-with-small onto cores: slot0 capacity C0 = max big count,
    slot1 capacity C1 = max small count (fewer padded PE columns than one
    global capacity).
  - Weights are quantized to cut HBM traffic (the kernel is memory-bound):
    w1 entirely in fp8 E3M4 (x2^8 scale), w2 rows 0..H/2 in E3M4 and rows
    H/2..H in bf16, both x2^9 so one PSUM accumulation chain has a uniform
    scale. Weights are uniform-distributed, so E3M4's 4 mantissa bits keep
    the end-to-end rel err ~1.6e-2 (< 2e-2 gate). x is pre-scaled by 2^-8
    on host (exact pow2 in bf16) so stage-1 PSUM is unscaled; the 2^-9
    dequant rides the stage-2 bias epilogue for free.
  - Device per core: stage1 hT = relu(w1q.T @ xT + b1) (d-outer first pass
    while w1 streams in, h-outer second pass on resident weights), stage2
    yT = (w2q.T @ hT) * 2^-9 + b2 with h-outer w2 streaming. Epilogues
    alternate ScalarE/VectorE. ~10 warmup matmuls run during the DMA
    lead-in so the PE's HAM clock-gate (cold 1.2 GHz) is released before
    real work starts.
  - All DMAs are per-partition contiguous (host pre-transposes to
    [P, ...]-major layouts): weight loads 4-16KB/partition/transfer.
    Outputs stream per 4-d-chunk group; slot0's outputs ride the gpsimd
    SWDGE queue so they never head-of-line block slot1's weight loads on
    the sync HWDGE queue.
"""

import numpy as np
import ml_dtypes

D = 1024
H = 2048
E = 16
NCORES = 8
P = 128
DB = D // P    # 8 d-chunks
HB = H // P    # 16 h-chunks
HB2 = HB // 2  # 8 h-chunks per w2 half
W1G = 2        # d-chunks per w1 DMA group (4 groups)
W2G = 4        # h-chunks per w2 DMA group (2 fp8 + 2 bf16 groups)
S1 = 8         # w1 scale exponent: w1q = w1 * 2^S1, x' = x * 2^-S1
S2 = 9         # w2 scale exponent: w2q = w2 * 2^S2, y = acc * 2^-S2 + b2
NWARM = 35     # PE warmup matmuls (~4.3us at cold 1.2 GHz; HAM needs ~3.4us)

_program_cache = {}


def _build_program(C0, C1):
    """Per-core Bass/Tile program; slot capacities C0 (big), C1 (small)."""
    import concourse.tile as tile
    from concourse import bacc, mybir

    assert C0 <= 512 and C1 <= 512
    f32 = mybir.dt.float32
    bf16 = mybir.dt.bfloat16
    f8 = mybir.dt.float8e3
    AF = mybir.ActivationFunctionType
    ALU = mybir.AluOpType
    CS = (C0, C1)

    nc = bacc.Bacc(
        "TRN2", target_bir_lowering=False, debug=False, num_devices=NCORES
    )
    xT = nc.dram_tensor("xT", [P, DB * (C0 + C1)], bf16, kind="ExternalInput").ap()
    w1q = nc.dram_tensor("w1q", [2, P, DB * H], f8, kind="ExternalInput").ap()
    w2f = nc.dram_tensor("w2f", [2, P, HB2 * D], f8, kind="ExternalInput").ap()
    w2b = nc.dram_tensor("w2b", [2, P, HB2 * D], bf16, kind="ExternalInput").ap()
    b1s = nc.dram_tensor("b1s", [2, P, HB], f32, kind="ExternalInput").ap()
    b2s = nc.dram_tensor("b2s", [2, P, DB], f32, kind="ExternalInput").ap()
    yT = nc.dram_tensor("yT", [P, DB * (C0 + C1)], bf16, kind="ExternalOutput").ap()

    with tile.TileContext(nc) as tc:
        with (
            tc.tile_pool(name="xp", bufs=2) as xp,
            tc.tile_pool(name="w1p", bufs=8) as w1p,
            tc.tile_pool(name="w2p", bufs=4) as w2p,
            tc.tile_pool(name="hp", bufs=32) as hp,
            tc.tile_pool(name="bp", bufs=4) as bp,
            tc.tile_pool(name="yp", bufs=4) as yp,
            tc.tile_pool(name="wm", bufs=1) as wm,
            tc.tile_pool(name="ps", bufs=8, space="PSUM") as ps,
        ):
            # Tiny bias tiles ride the gpsimd SWDGE queue.
            b1ts, b2ts = [], []
            for s in range(2):
                b1t = bp.tile([P, HB], f32, tag="b1")
                nc.gpsimd.dma_start(b1t[:], b1s[s])
                b1ts.append(b1t)
                b2t = bp.tile([P, DB], f32, tag="b2")
                nc.gpsimd.dma_start(b2t[:], b2s[s])
                b2ts.append(b2t)

            # PE warmup during the DMA lead-in: the HAM clock gate holds the
            # PE at 1.2 GHz until it sees ~3.4us of sustained activity.
            wt = wm.tile([P, P + C0], bf16, tag="warm")
            nc.vector.memset(wt[:], 1.0)
            wps = ps.tile([P, C0], f32, tag="acc", name="warmps")
            for _ in range(NWARM):
                nc.tensor.matmul(
                    wps[:], lhsT=wt[:, 0:P], rhs=wt[:, P:P + C0],
                    start=True, stop=True,
                )

            def epi1(i, out_t, acc_t, bias_col):
                """relu(acc + b1), alternating engines."""
                if i % 2 == 0:
                    nc.scalar.activation(out_t, acc_t, AF.Relu, bias=bias_col)
                else:
                    nc.vector.tensor_scalar(
                        out_t, acc_t, bias_col, 0.0, ALU.add, ALU.max
                    )

            def epi2(i, out_t, acc_t, bias_col):
                """acc * 2^-S2 + b2, alternating engines."""
                if i % 2 == 0:
                    nc.scalar.activation(
                        out_t, acc_t, AF.Identity, bias=bias_col,
                        scale=float(2.0 ** -S2),
                    )
                else:
                    nc.vector.tensor_scalar(
                        out_t, acc_t, float(2.0 ** -S2), bias_col,
                        ALU.mult, ALU.add,
                    )

            xoff = 0
            for s in range(2):
                Cs = CS[s]
                # x + w1 for this slot (sync HWDGE queue, consumption order)
                xt = xp.tile([P, DB * Cs], bf16, tag="xT", name=f"x_{s}")
                nc.sync.dma_start(xt[:], xT[:, xoff:xoff + DB * Cs])
                w1ts = []
                for g in range(DB // W1G):
                    w1t = w1p.tile([P, W1G * H], f8, tag="w1", name=f"w1_{s}_{g}")
                    nc.sync.dma_start(
                        w1t[:], w1q[s][:, g * W1G * H:(g + 1) * W1G * H]
                    )
                    w1ts.append(w1t)

                def w1col(d, h):
                    return w1ts[d // W1G][:, (d % W1G) * H + h * P:
                                         (d % W1G) * H + h * P + P]

                # ---- stage 1: hT = relu(w1q.T @ xT + b1) ----
                hts = [None] * HB
                # pass 0 (h 0..7): d-outer so the PE starts on w1 group 0 as
                # soon as it lands and tracks the stream.
                accs1 = [
                    ps.tile([P, Cs], f32, tag="acc", name=f"a1_{s}_{i}")
                    for i in range(HB2)
                ]
                for d in range(DB):
                    xd = xt[:, d * Cs:(d + 1) * Cs]
                    for h in range(HB2):
                        nc.tensor.matmul(
                            accs1[h][:], lhsT=w1col(d, h), rhs=xd,
                            start=(d == 0), stop=(d == DB - 1),
                        )
                for h in range(HB2):
                    ht = hp.tile([P, Cs], bf16, tag="hT", name=f"h_{s}_{h}")
                    epi1(h, ht[:], accs1[h][:], b1ts[s][:, h:h + 1])
                    hts[h] = ht
                # pass 1 (h 8..15): weights resident -> h-outer, retire each
                # psum immediately so epilogues pipeline with next chain.
                for h in range(HB2, HB):
                    acc = ps.tile([P, Cs], f32, tag="acc", name=f"a1b_{s}_{h}")
                    for d in range(DB):
                        nc.tensor.matmul(
                            acc[:], lhsT=w1col(d, h),
                            rhs=xt[:, d * Cs:(d + 1) * Cs],
                            start=(d == 0), stop=(d == DB - 1),
                        )
                    ht = hp.tile([P, Cs], bf16, tag="hT", name=f"h_{s}_{h}")
                    epi1(h, ht[:], acc[:], b1ts[s][:, h:h + 1])
                    hts[h] = ht

                # ---- stage 2: yT = (w2q.T @ hT) * 2^-S2 + b2 ----
                # h-outer: each w2 group is consumed as soon as its DMA lands.
                accs2 = [
                    ps.tile([P, Cs], f32, tag="acc", name=f"a2_{s}_{d}")
                    for d in range(DB)
                ]
                for g in range(HB // W2G):
                    fp8_half = g < HB2 // W2G
                    w2t = w2p.tile(
                        [P, W2G * D], f8 if fp8_half else bf16,
                        tag="w2f" if fp8_half else "w2b", name=f"w2_{s}_{g}",
                    )
                    src = (w2f if fp8_half else w2b)[s]
                    go = (g % (HB2 // W2G)) * W2G * D
                    nc.sync.dma_start(w2t[:], src[:, go:go + W2G * D])
                    for hh in range(W2G):
                        h = g * W2G + hh
                        for d in range(DB):
                            nc.tensor.matmul(
                                accs2[d][:],
                                lhsT=w2t[:, hh * D + d * P:hh * D + d * P + P],
                                rhs=hts[h][:],
                                start=(h == 0), stop=(h == HB - 1),
                            )
                # Stream outputs per 4-d-chunk half. slot0 goes via gpsimd so
                # it can't head-of-line block slot1's weight loads on sync.
                for half in range(2):
                    yt = yp.tile([P, 4 * Cs], bf16, tag="yt",
                                 name=f"y_{s}_{half}")
                    for j in range(4):
                        d = half * 4 + j
                        epi2(d, yt[:, j * Cs:(j + 1) * Cs], accs2[d][:],
                             b2ts[s][:, d:d + 1])
                    dst = yT[:, xoff + half * 4 * Cs:
                             xoff + half * 4 * Cs + 4 * Cs]
                    (nc.gpsimd if s == 0 else nc.sync).dma_start(dst, yt[:])
                xoff += DB * Cs

    nc.compile()
    return nc


def _ceil4(n):
    return max(32, (int(n) + 3) // 4 * 4)


def kernel(x, gate_w, gate_b, w1, b1, w2, b2, _trace=False):
    from concourse.bass_utils import run_bass_kernel_spmd

    x = np.asarray(x, dtype=np.float32)
    B, S, d_in = x.shape
    T = B * S
    xf = x.reshape(T, d_in)

    # --- routing (host side: the dispatch/sharding step) ---
    logits = xf @ np.asarray(gate_w, dtype=np.float32) + np.asarray(
        gate_b, dtype=np.float32
    )
    top1 = np.argmax(logits, axis=-1)
    idxs = [np.nonzero(top1 == e)[0] for e in range(E)]
    counts = np.array([len(i) for i in idxs])
    order = np.argsort(-counts, kind="stable")
    slot0_ids = order[:NCORES]              # big experts, one per core
    slot1_ids = order[NCORES:][::-1]        # paired smallest-with-biggest
    C0 = _ceil4(counts[slot0_ids].max())
    C1 = _ceil4(counts[slot1_ids].max())
    C0, C1 = min(C0, 512), min(C1, 512)
    assert counts[slot0_ids].max() <= C0 and counts[slot1_ids].max() <= C1, (
        "expert capacity overflow"
    )

    if (C0, C1) not in _program_cache:
        _program_cache[(C0, C1)] = _build_program(C0, C1)
    nc = _program_cache[(C0, C1)]

    bf16 = ml_dtypes.bfloat16
    e3m4 = ml_dtypes.float8_e3m4
    w1 = np.asarray(w1, dtype=np.float32)
    w2 = np.asarray(w2, dtype=np.float32)
    b1 = np.asarray(b1, dtype=np.float32)
    b2 = np.asarray(b2, dtype=np.float32)

    # Quantize all experts at once, in the [P, ...]-major DMA layouts.
    # w1 row d = o*P + p  ->  w1qh[e, p, o*H:(o+1)*H]
    w1qh = np.ascontiguousarray(
        (w1 * 2.0 ** S1).astype(e3m4).reshape(E, DB, P, H).transpose(0, 2, 1, 3)
    ).reshape(E, P, DB * H)
    w2s = w2 * 2.0 ** S2
    w2fh = np.ascontiguousarray(
        w2s[:, :H // 2].astype(e3m4).reshape(E, HB2, P, D).transpose(0, 2, 1, 3)
    ).reshape(E, P, HB2 * D)
    w2bh = np.ascontiguousarray(
        w2s[:, H // 2:].astype(bf16).reshape(E, HB2, P, D).transpose(0, 2, 1, 3)
    ).reshape(E, P, HB2 * D)
    b1h = b1.reshape(E, HB, P).transpose(0, 2, 1)   # [E, P, HB]
    b2h = b2.reshape(E, DB, P).transpose(0, 2, 1)   # [E, P, DB]

    xs = (xf * 2.0 ** -S1).astype(bf16)

    in_maps = []
    pair = [(int(slot0_ids[c]), int(slot1_ids[c])) for c in range(NCORES)]
    for core in range(NCORES):
        xT = np.zeros((P, DB * (C0 + C1)), dtype=bf16)
        off = 0
        for s, Cs in ((0, C0), (1, C1)):
            e = pair[core][s]
            idx = idxs[e]
            if len(idx):
                # [len, D] -> [P, DB, len]
                blk = xs[idx].T.reshape(DB, P, len(idx)).transpose(1, 0, 2)
                xv = xT[:, off:off + DB * Cs].reshape(P, DB, Cs)
                xv[:, :, :len(idx)] = blk
            off += DB * Cs
        es = [pair[core][0], pair[core][1]]
        in_maps.append({
            "xT": xT,
            "w1q": w1qh[es],
            "w2f": w2fh[es],
            "w2b": w2bh[es],
            "b1s": np.ascontiguousarray(b1h[es]),
            "b2s": np.ascontiguousarray(b2h[es]),
        })

    res = run_bass_kernel_spmd(
        nc, in_maps, core_ids=list(range(NCORES)), trace=_trace
    )

    out = np.zeros((T, D), dtype=np.float32)
    for core in range(NCORES):
        yT_out = res.results[core]["yT"]
        off = 0
        for s, Cs in ((0, C0), (1, C1)):
            e = pair[core][s]
            idx = idxs[e]
            if len(idx):
                blk = yT_out[:, off:off + DB * Cs].reshape(P, DB, Cs)
                # [P, DB, Cs] -> [D, Cs] with row d = o*P + p
                yD = blk.transpose(1, 0, 2).reshape(D, Cs)
                out[idx] = yD[:, :len(idx)].T.astype(np.float32)
            off += DB * Cs
    if _trace:
        kernel.last_result = res
    return out.reshape(B, S, D)
